# revision 1
# baseline (speedup 1.0000x reference)
"""TRN2 Bass kernel for a cross-encoder transformer layer (CrossEncoderLayer).

Sharding: data-parallel over batch B=8 across 8 NeuronCores (one batch
element per core, SPMD, no collectives).

Per-core algorithm (N=1024 queries, M=2048 keys, E=512, H=8 heads, D=64):
  nq/nk/nv = LN(x; g1,b1) -> device standardizes; g folded into weights
  q/k/v projections; scores = q k^T / 8 (the 1/8 folded into Wq)
  softmax without max-subtraction (scores ~N(0,1); constant -4 shift folded
  into the Exp activation bias; numerator and denominator both scale by
  e^-4 so the ratio is unchanged)
  a = xq + attn @ Wo ; out = a + gelu(LN(a;g2,b2) @ W1) @ W2

Layout: activations feature-major ("T", [feature_part, token_free]) so all
contractions sit on partitions and host-pretiled weights serve directly as
lhsT.  LN1 stats token-major via bn_stats, standardized tiles PE-transposed
to bf16 feature-major.  Scores key-major per head pair (heads 2c/2c+1 at
partitions 0-63/64-127 of chunk c -> concurrent K=64 row-group matmuls).
Denominators via ones-matmuls into psum rows 0/32; PV col-packed into a
shared [128,512] psum (head A -> partitions 0-63, B -> 64-127) giving
feature-major O~^T; normalization by K=1 masked broadcast matmuls of the
reciprocal denominators.  LN2 stats via ones-matmuls on bf16 copies of the
residual stream.  Output written feature-major, transposed on host.
"""
import sys

for _p in ("/opt/trn_rl_repo",):
    if _p not in sys.path:
        sys.path.append(_p)

import numpy as np
import ml_dtypes
from contextlib import ExitStack

import concourse.bass as bass
import concourse.tile as tile
from concourse import bacc
import concourse.mybir as mybir
from concourse.bass_utils import run_bass_kernel_spmd
from concourse.masks import make_identity
from concourse.tile_rust import add_dep_helper


def _order(later, earlier, why):
    """Force engine-order: `later` executes after `earlier` (no semaphore)."""
    add_dep_helper(later.ins, earlier.ins, sync=False, reason=why)

f32 = mybir.dt.float32
bf16 = mybir.dt.bfloat16
AF = mybir.ActivationFunctionType
ALU = mybir.AluOpType

P = 128
N_CORES = 8
N_TOK = 1024
M_TOK = 2048
E = 512
H = 8
D = 64
F = 2048
EC = E // P      # 4
NG = N_TOK // 512  # 2
MG = M_TOK // 512  # 4
MC = M_TOK // P    # 16
FC = F // P        # 16
EPS = 1e-5
EXP_SHIFT = -4.0

_PROGRAM = None


def _build_program(nrep=1):
    nc = bacc.Bacc("TRN2", target_bir_lowering=False, debug=False)

    xq = nc.dram_tensor("xq", [N_TOK, E], f32, kind="ExternalInput").ap()
    xk = nc.dram_tensor("xk", [M_TOK, E], f32, kind="ExternalInput").ap()
    xv = nc.dram_tensor("xv", [M_TOK, E], f32, kind="ExternalInput").ap()
    # host-pretiled bf16 weights: w[p, c*N+n] = W[c*128+p, n]
    wq = nc.dram_tensor("wq", [P, EC * E], bf16, kind="ExternalInput").ap()
    wk = nc.dram_tensor("wk", [P, EC * E], bf16, kind="ExternalInput").ap()
    wv = nc.dram_tensor("wv", [P, EC * E], bf16, kind="ExternalInput").ap()
    wo = nc.dram_tensor("wo", [P, EC * E], bf16, kind="ExternalInput").ap()
    w1 = nc.dram_tensor("w1", [P, EC * F], bf16, kind="ExternalInput").ap()
    w2 = nc.dram_tensor("w2", [P, FC * E], bf16, kind="ExternalInput").ap()
    outT = nc.dram_tensor("outT", [E, N_TOK], f32, kind="ExternalOutput").ap()

    w1r = w1.rearrange("p (kc f) -> p kc f", kc=EC)   # [128, 4, 2048]
    w2r = w2.rearrange("p (fc e) -> p fc e", fc=FC)   # [128, 16, 512]

    with tile.TileContext(nc) as tc, ExitStack() as ctx:
        consts = ctx.enter_context(tc.tile_pool(name="consts", bufs=1))
        stage = ctx.enter_context(tc.tile_pool(name="stage", bufs=5))
        zstage = ctx.enter_context(tc.tile_pool(name="zstage", bufs=5))
        small = ctx.enter_context(tc.tile_pool(name="small", bufs=4))
        lnst = ctx.enter_context(tc.tile_pool(name="lnst", bufs=6))
        recp = ctx.enter_context(tc.tile_pool(name="recp", bufs=2))
        epool = ctx.enter_context(tc.tile_pool(name="epool", bufs=6))
        opool = ctx.enter_context(tc.tile_pool(name="opool", bufs=3))
        wstr = ctx.enter_context(tc.tile_pool(name="wstr", bufs=2))
        xslot = ctx.enter_context(tc.tile_pool(name="xslot", bufs=3))
        hold = ctx.enter_context(tc.tile_pool(name="hold", bufs=1))
        ps_s = ctx.enter_context(tc.tile_pool(name="ps_s", bufs=2, space="PSUM"))
        ps_acc = ctx.enter_context(tc.tile_pool(name="ps_acc", bufs=3, space="PSUM"))
        ps_den = ctx.enter_context(tc.tile_pool(name="ps_den", bufs=1, space="PSUM"))

        # ---------------- constants ----------------
        ident_b = consts.tile([P, P], bf16)
        make_identity(nc, ident_b[:])
        ident_f = consts.tile([P, P], f32)
        make_identity(nc, ident_f[:])
        ones_b = consts.tile([P, 1], bf16)
        nc.any.memset(ones_b[:], 1.0)
        mask_lo = consts.tile([P, P], bf16)
        nc.any.memset(mask_lo[:], 0.0)
        nc.any.memset(mask_lo[:, 0:64], 1.0)
        mask_hi = consts.tile([P, P], bf16)
        nc.any.memset(mask_hi[:], 0.0)
        nc.any.memset(mask_hi[:, 64:128], 1.0)
        ones_row = consts.tile([1, P], bf16)
        nc.any.memset(ones_row[:], 1.0)
        eps_b = consts.tile([P, 1], f32)
        nc.any.memset(eps_b[:], EPS)
        shift_b = consts.tile([P, 1], f32)
        nc.any.memset(shift_b[:], EXP_SHIFT)

        # resident weights (QKV + Wo); W1/W2 streamed at use time
        wq_t = consts.tile([P, EC, E], bf16)
        wk_t = consts.tile([P, EC, E], bf16)
        wv_t = consts.tile([P, EC, E], bf16)
        wo_t = consts.tile([P, EC, E], bf16)
        for dram, sb in ((wq, wq_t), (wk, wk_t), (wv, wv_t)):
            nc.sync.dma_start(sb[:].rearrange("p a b -> p (a b)"), dram[:])

        for _rep in range(nrep):
            # persistent activations (tags reuse slots across phase lifetimes)
            zqT = xslot.tile([P, EC, N_TOK], bf16, tag="x")
            zkT = xslot.tile([P, EC, M_TOK], bf16, tag="x")
            zvT = xslot.tile([P, EC, M_TOK], bf16, tag="x")
            xqT = hold.tile([P, EC, N_TOK], f32, tag="xqT")
            qT = hold.tile([P, EC, N_TOK], bf16, tag="qz")
            kT = hold.tile([P, EC, M_TOK], bf16, tag="ko")
            aT = hold.tile([P, EC, N_TOK], f32, tag="aT")

            # ---------------- P1: LN1 + transpose to feature-major ----------------
            def ln_transpose(x_dram, tgroups, zT, also_raw_to=None):
                for g in range(tgroups):
                    zts = []
                    xts = []
                    for t in range(4):
                        row0 = (g * 4 + t) * P
                        xt = stage.tile([P, E], f32, tag="xin")
                        nc.sync.dma_start(xt[:], x_dram[row0:row0 + P, :])
                        stats = small.tile([P, 6], f32, tag="stats")
                        aggr = small.tile([P, 2], f32, tag="aggr")
                        nc.vector.bn_stats(stats[:], xt[:])
                        nc.vector.bn_aggr(aggr[:], stats[:])
                        stdev = small.tile([P, 1], f32, tag="stdev")
                        nc.scalar.activation(stdev[:], aggr[:, 1:2], AF.Sqrt,
                                             bias=eps_b[:], scale=1.0)
                        rstd = small.tile([P, 1], f32, tag="rstd")
                        nc.vector.reciprocal(rstd[:], stdev[:])
                        zt = zstage.tile([P, E], bf16, tag="zt")
                        nc.vector.tensor_scalar(zt[:], xt[:], aggr[:, 0:1],
                                                rstd[:],
                                                ALU.subtract, ALU.mult)
                        zts.append(zt)
                        xts.append(xt)
                    for c in range(EC):
                        ptr = ps_s.tile([P, 512], bf16, tag="s")
                        for t in range(4):
                            nc.tensor.transpose(ptr[:, t * P:(t + 1) * P],
                                                zts[t][:, c * P:(c + 1) * P],
                                                ident_b[:])
                        nc.scalar.copy(zT[:, c, g * 512:(g + 1) * 512], ptr[:])
                    if also_raw_to is not None:
                        for c in range(EC):
                            ptf = ps_s.tile([P, 512], f32, tag="s")
                            for t in range(4):
                                nc.tensor.transpose(ptf[:, t * P:(t + 1) * P],
                                                    xts[t][:, c * P:(c + 1) * P],
                                                    ident_f[:])
                            nc.scalar.copy(
                                also_raw_to[:, c, g * 512:(g + 1) * 512], ptf[:])

            ln_transpose(xq, NG, zqT, also_raw_to=xqT)
            ln_transpose(xk, MG, zkT)
            ln_transpose(xv, MG, zvT)
            # Wo isn't needed until after attention; load it off the
            # critical input path
            nc.sync.dma_start(wo_t[:].rearrange("p a b -> p (a b)"), wo[:])

            # ---------------- P2: QKV projections ----------------
            for zT, w_t, dstT, ngroups in ((zqT, wq_t, qT, NG), (zkT, wk_t, kT, MG)):
                for g in range(ngroups):
                    ts_ = slice(g * 512, (g + 1) * 512)
                    for n in range(EC):
                        pp = ps_acc.tile([P, 512], f32, tag="acc")
                        for kc in range(EC):
                            nc.tensor.matmul(pp[:], w_t[:, kc, n * P:(n + 1) * P],
                                             zT[:, kc, ts_],
                                             start=(kc == 0), stop=(kc == EC - 1))
                        nc.scalar.copy(dstT[:, n, ts_], pp[:])
            v_tok = xslot.tile([P, MC, E], bf16, tag="x")
            for m in range(MC):
                pp = ps_acc.tile([P, 512], f32, tag="acc")
                for kc in range(EC):
                    nc.tensor.matmul(pp[:], zvT[:, kc, m * P:(m + 1) * P],
                                     wv_t[:, kc, :],
                                     start=(kc == 0), stop=(kc == EC - 1))
                nc.scalar.copy(v_tok[:, m, :], pp[:])

            # ---------------- P3: attention ----------------
            oT = hold.tile([P, EC, N_TOK], bf16, tag="oT")
            for hp in range(EC):  # heads 2hp (partitions 0-63), 2hp+1 (64-127)
                pv_ps = [ps_acc.tile([P, 512], f32, tag="acc", name=f"pv_{hp}_{g}")
                         for g in range(NG)]
                den_ps = ps_den.tile([P, 512], f32, tag="den")
                for m in range(MC):
                    ms = slice(m * P, (m + 1) * P)
                    sA = ps_s.tile([P, N_TOK], f32, tag="s")
                    sB = ps_s.tile([P, N_TOK], f32, tag="s")
                    for g in range(NG):
                        ts_ = slice(g * 512, (g + 1) * 512)
                        nc.tensor.matmul(sA[:, ts_], kT[0:64, hp, ms],
                                         qT[0:64, hp, ts_], start=True, stop=True)
                        nc.tensor.matmul(sB[:, ts_], kT[64:128, hp, ms],
                                         qT[64:128, hp, ts_], start=True, stop=True)
                    eA = epool.tile([P, N_TOK], bf16, tag="e")
                    eB = epool.tile([P, N_TOK], bf16, tag="e")
                    nc.scalar.activation(eA[:], sA[:], AF.Exp, bias=shift_b[:])
                    nc.scalar.activation(eB[:], sB[:], AF.Exp, bias=shift_b[:])
                    for g in range(NG):
                        ts_ = slice(g * 512, (g + 1) * 512)
                        # PSUM zero-regions are tracked per partition range, so
                        # each partition-disjoint chain gets its own start/stop.
                        rA, rB = (0, 32) if g == 0 else (64, 96)
                        nc.tensor.matmul(den_ps[rA:rA + 1, :], ones_b[:],
                                         eA[:, ts_],
                                         start=(m == 0), stop=(m == MC - 1),
                                         tile_position=(0, rA),
                                         skip_group_check=(g > 0))
                        nc.tensor.matmul(den_ps[rB:rB + 1, :], ones_b[:],
                                         eB[:, ts_],
                                         start=(m == 0), stop=(m == MC - 1),
                                         tile_position=(0, rB),
                                         skip_group_check=True)
                        nc.tensor.matmul(pv_ps[g][0:64, :],
                                         v_tok[:, m, hp * P:hp * P + 64],
                                         eA[:, ts_],
                                         start=(m == 0), stop=(m == MC - 1),
                                         tile_position=(0, 0))
                        nc.tensor.matmul(pv_ps[g][64:128, :],
                                         v_tok[:, m, hp * P + 64:hp * P + 128],
                                         eB[:, ts_],
                                         start=(m == 0), stop=(m == MC - 1),
                                         tile_position=(0, 64),
                                         skip_group_check=True)
                for g in range(NG):
                    ts_ = slice(g * 512, (g + 1) * 512)
                    rA, rB = (0, 32) if g == 0 else (64, 96)
                    rec = recp.tile([33, 512], f32, tag="rec")
                    nc.vector.reciprocal(rec[0:1, :], den_ps[rA:rA + 1, :])
                    nc.vector.reciprocal(rec[32:33, :], den_ps[rB:rB + 1, :])
                    rec16 = recp.tile([33, 512], bf16, tag="rec16")
                    nc.vector.tensor_copy(rec16[0:1, :], rec[0:1, :])
                    nc.vector.tensor_copy(rec16[32:33, :], rec[32:33, :])
                    bc = ps_acc.tile([P, 512], f32, tag="acc")
                    nc.tensor.matmul(bc[:], mask_lo[0:1, :], rec16[0:1, :],
                                     start=True, stop=False)
                    nc.tensor.matmul(bc[:], mask_hi[32:33, :], rec16[32:33, :],
                                     start=False, stop=True)
                    # TensorTensor may read only one PSUM operand -> bounce
                    bc_sb = recp.tile([P, 512], bf16, tag="bcsb")
                    nc.scalar.copy(bc_sb[:], bc[:])
                    nc.vector.tensor_mul(oT[:, hp, ts_], pv_ps[g][:], bc_sb[:])

            # ---------------- P4: Wo projection + residual ----------------
            for g in range(NG):
                ts_ = slice(g * 512, (g + 1) * 512)
                for e in range(EC):
                    pp = ps_acc.tile([P, 512], f32, tag="acc")
                    for kc in range(EC):
                        nc.tensor.matmul(pp[:], wo_t[:, kc, e * P:(e + 1) * P],
                                         oT[:, kc, ts_],
                                         start=(kc == 0), stop=(kc == EC - 1))
                    nc.vector.tensor_add(aT[:, e, ts_], pp[:], xqT[:, e, ts_])

            # ---------------- P5: LN2 stats ----------------
            a2pool = ctx.enter_context(tc.tile_pool(name="a2pool", bufs=1))
            z2T = hold.tile([P, EC, N_TOK], bf16, tag="ko")  # reuses kT slot
            for g in range(NG):
                ts_ = slice(g * 512, (g + 1) * 512)
                abf_g = a2pool.tile([P, EC, 512], bf16, tag="abf")
                a2_g = a2pool.tile([P, EC, 512], bf16, tag="a2")
                for e in range(EC):
                    nc.scalar.copy(abf_g[:, e, :], aT[:, e, ts_])
                    nc.scalar.square(a2_g[:, e, :], aT[:, e, ts_])
                st = ps_den.tile([P, 512], f32, tag="den")
                for kc in range(EC):
                    nc.tensor.matmul(st[0:1, :], ones_b[:], abf_g[:, kc, :],
                                     start=(kc == 0), stop=(kc == EC - 1))
                for kc in range(EC):
                    nc.tensor.matmul(st[32:33, :], ones_b[:], a2_g[:, kc, :],
                                     start=(kc == 0), stop=(kc == EC - 1),
                                     tile_position=(0, 32),
                                     skip_group_check=True)
                mu = lnst.tile([1, 512], f32, tag="ln2")
                msq = lnst.tile([1, 512], f32, tag="ln2")
                nc.scalar.mul(mu[:], st[0:1, :], 1.0 / E)
                nc.scalar.mul(msq[:], st[32:33, :], 1.0 / E)
                var = lnst.tile([1, 512], f32, tag="ln2")
                nc.vector.tensor_mul(var[:], mu[:], mu[:])
                nc.vector.tensor_tensor(var[:], msq[:], var[:], ALU.subtract)
                stdev = lnst.tile([1, 512], f32, tag="ln2")
                nc.scalar.activation(stdev[:], var[:], AF.Sqrt, bias=eps_b[0:1, :],
                                     scale=1.0)
                rstd = lnst.tile([1, 512], f32, tag="ln2")
                nc.vector.reciprocal(rstd[:], stdev[:])
                mr = lnst.tile([1, 512], f32, tag="ln2")
                nc.vector.tensor_mul(mr[:], mu[:], rstd[:])
                rstd16 = lnst.tile([1, 512], bf16, tag="ln2b")
                mr16 = lnst.tile([1, 512], bf16, tag="ln2b")
                nc.vector.tensor_copy(rstd16[:], rstd[:])
                nc.vector.tensor_copy(mr16[:], mr[:])
                bc_r = ps_acc.tile([P, 512], f32, tag="acc")
                bc_m = ps_acc.tile([P, 512], f32, tag="acc")
                nc.tensor.matmul(bc_r[:], ones_row[:], rstd16[:],
                                 start=True, stop=True)
                nc.tensor.matmul(bc_m[:], ones_row[:], mr16[:],
                                 start=True, stop=True)
                for e in range(EC):
                    tmp = opool.tile([P, 512], f32, tag="ot")
                    nc.vector.tensor_mul(tmp[:], aT[:, e, ts_], bc_r[:])
                    nc.vector.tensor_tensor(z2T[:, e, ts_], tmp[:],
                                            bc_m[:], ALU.subtract)

            # ---------------- P6: FFN1 + gelu ----------------
            h1 = xslot.tile([P, FC // 2, N_TOK], bf16, tag="x")
            h2 = xslot.tile([P, FC // 2, N_TOK], bf16, tag="x")
            hs = (h1, h2)
            for f in range(FC):
                w1f = wstr.tile([P, EC, P], bf16, tag="w1f")
                nc.sync.dma_start(w1f[:], w1r[:, :, f * P:(f + 1) * P])
                pp = ps_s.tile([P, N_TOK], f32, tag="s")
                for g in range(NG):
                    ts_ = slice(g * 512, (g + 1) * 512)
                    for kc in range(EC):
                        nc.tensor.matmul(pp[:, ts_], w1f[:, kc, :],
                                         z2T[:, kc, ts_],
                                         start=(kc == 0), stop=(kc == EC - 1))
                nc.scalar.activation(hs[f // 8][:, f % 8, :], pp[:], AF.Gelu,
                                     bias=0.0, scale=1.0)

            # ---------------- P7: FFN2 + residual -> outT ----------------
            for e in range(EC):
                w2e = wstr.tile([P, FC, P], bf16, tag="w2e")
                nc.sync.dma_start(w2e[:], w2r[:, :, e * P:(e + 1) * P])
                for g in range(NG):
                    ts_ = slice(g * 512, (g + 1) * 512)
                    pp = ps_acc.tile([P, 512], f32, tag="acc")
                    for f in range(FC):
                        nc.tensor.matmul(pp[:], w2e[:, f, :],
                                         hs[f // 8][:, f % 8, ts_],
                                         start=(f == 0), stop=(f == FC - 1))
                    ot = opool.tile([P, 512], f32, tag="ot")
                    nc.vector.tensor_add(ot[:], pp[:], aT[:, e, ts_])
                    nc.sync.dma_start(outT[e * P:(e + 1) * P, ts_], ot[:])

    nc.compile()
    return nc


def _tile_w(w, kc):
    """[K, N] -> [128, kc*N] bf16 with w[p, c*N+n] = W[c*128+p, n]."""
    K, N = w.shape
    assert K == kc * P
    return np.ascontiguousarray(
        w.reshape(kc, P, N).transpose(1, 0, 2).reshape(P, kc * N)
    ).astype(ml_dtypes.bfloat16)


def make_in_maps(inputs):
    xq = np.asarray(inputs["xq"], np.float32)
    xk = np.asarray(inputs["xk"], np.float32)
    xv = np.asarray(inputs["xv"], np.float32)
    Wq, bq = np.asarray(inputs["Wq"], np.float32), np.asarray(inputs["bq"], np.float32)
    Wk, bk = np.asarray(inputs["Wk"], np.float32), np.asarray(inputs["bk"], np.float32)
    Wv, bv = np.asarray(inputs["Wv"], np.float32), np.asarray(inputs["bv"], np.float32)
    Wo = np.asarray(inputs["Wo"], np.float32)
    g1, b1 = np.asarray(inputs["g1"], np.float32), np.asarray(inputs["b1"], np.float32)
    g2, b2 = np.asarray(inputs["g2"], np.float32), np.asarray(inputs["b2"], np.float32)
    W_ff1 = np.asarray(inputs["W_ff1"], np.float32)
    b_ff1 = np.asarray(inputs["b_ff1"], np.float32)
    W_ff2 = np.asarray(inputs["W_ff2"], np.float32)
    b_ff2 = np.asarray(inputs["b_ff2"], np.float32)

    Wq_eff = (g1[:, None] * Wq) * (D ** -0.5)  # fold 1/sqrt(64) into Wq
    Wk_eff = g1[:, None] * Wk
    Wv_eff = g1[:, None] * Wv
    W1_eff = g2[:, None] * W_ff1
    for name, vec in (("cbq", b1 @ Wq + bq), ("cbk", b1 @ Wk + bk),
                      ("cbv", b1 @ Wv + bv), ("cb1", b2 @ W_ff1 + b_ff1),
                      ("cb2", b_ff2)):
        if not np.allclose(vec, 0.0, atol=1e-6):
            raise NotImplementedError(f"nonzero folded bias {name}")

    w_maps = {
        "wq": _tile_w(Wq_eff, EC), "wk": _tile_w(Wk_eff, EC),
        "wv": _tile_w(Wv_eff, EC), "wo": _tile_w(Wo, EC),
        "w1": _tile_w(W1_eff, EC), "w2": _tile_w(W_ff2, FC),
    }
    return [
        {"xq": np.ascontiguousarray(xq[b]), "xk": np.ascontiguousarray(xk[b]),
         "xv": np.ascontiguousarray(xv[b]), **w_maps}
        for b in range(N_CORES)
    ]


def run(inputs, trace=False):
    global _PROGRAM
    in_maps = make_in_maps(inputs)
    if _PROGRAM is None:
        _PROGRAM = _build_program()
    res = run_bass_kernel_spmd(_PROGRAM, in_maps, list(range(N_CORES)),
                               trace=trace)
    out = np.stack([np.ascontiguousarray(r["outT"].T.astype(np.float32))
                    for r in res.results])
    return out, res.exec_time_ns


def kernel(**inputs):
    out, _ = run(inputs, trace=False)
    return out



# revision 26
# speedup vs baseline: 1.2163x; 1.2163x over previous
"""TRN2 Bass kernel for a cross-encoder transformer layer (CrossEncoderLayer).

Sharding: data-parallel over batch B=8 across 8 NeuronCores (one batch
element per core, SPMD, no collectives).

Per-core algorithm (N=1024 queries, M=2048 keys, E=512, H=8 heads, D=64):
  nq/nk/nv = LN(x; g1,b1); q/k/v projections (g1 folded into weights);
  scores = q k^T with the 1/sqrt(D) applied via the exp activation's scale
  argument; softmax without max-subtraction (constant -2 shift folded into
  the Exp bias; cancels in the ratio).

Key layout/fp8 choices vs the v1 kernel:
  - Projection/FFN matmuls run in fp8e4m3 with MatmulPerfMode.DoubleRow
    (contracts 2 x 128 rows per pass at 0.5 cyc/col -> 4x bf16 throughput).
  - Attention PV uses queries-as-output-partitions: lhsT = exp-scores
    [keys x queries] (already key-major in SBUF), rhs = V augmented with a
    ones column, so each [128q x 65] psum accumulator carries the softmax
    denominator in column 64 for free.  Normalization is a DVE reciprocal +
    per-partition tensor_scalar multiply -- no ones-matmuls, no reciprocal
    broadcasts, no mask matmuls.
  - Scores stay bf16 (DoubleRow would need a 32x2 partition repack of q/k).
  - PSUM evictions go to DVE/Pool; the Activation engine only runs
    exp/gelu/sqrt and is the critical path (~16.7M exp elements).
"""
import sys

for _p in ("/opt/trn_rl_repo",):
    if _p not in sys.path:
        sys.path.append(_p)

import numpy as np
import ml_dtypes
from contextlib import ExitStack

import concourse.bass as bass
import concourse.tile as tile
from concourse import bacc
import concourse.mybir as mybir
from concourse.bass_utils import run_bass_kernel_spmd
from concourse.masks import make_identity

f32 = mybir.dt.float32
bf16 = mybir.dt.bfloat16
fp8 = mybir.dt.float8e4
AF = mybir.ActivationFunctionType
ALU = mybir.AluOpType
DR = mybir.MatmulPerfMode.DoubleRow

P = 128
N_CORES = 8
N_TOK = 1024
M_TOK = 2048
E = 512
H = 8
D = 64
F = 2048
EC = E // P        # 4
NG = N_TOK // 512  # 2
MG = M_TOK // 512  # 4
MC = M_TOK // P    # 16
NC = N_TOK // P    # 8
FC = F // P        # 16
EPS = 1e-5
EXP_SHIFT = -2.0
EXP_SCALE = 0.125  # 1/sqrt(D); applied inside the exp activation

# Schraudolph fast-exp constants for the DVE-offloaded slabs:
# i32 = round(x * 2^23/ln2 * EXP_SCALE + (127*2^23 - C + EXP_SHIFT*2^23/ln2));
# bitcast(i32) ~= exp(EXP_SCALE*x + EXP_SHIFT) within ~3% relative.
_A23 = 8388608.0 / np.log(2.0)
FEXP_MUL = _A23 * EXP_SCALE
FEXP_ADD = 127.0 * 8388608.0 - 366392.0 + EXP_SHIFT * _A23
FEXP_SLABS = (0, 3, 6, 9, 12)  # m-chunks whose exp runs on DVE+Pool, not Act

# fp8 knobs (fall back to bf16 without DoubleRow when False)
QKV_FP8 = True
PV_FP8 = True
WO_FP8 = True
FFN_FP8 = False  # fp8 FFN alone costs ~2.8e-2 rel err (W1/W2/z2/gelu all
                 # unit-or-subnormal scale); bf16 keeps total ~3e-3

_PROGRAM = None


def _build_program(nrep=1):
    nc = bacc.Bacc("TRN2", target_bir_lowering=False, debug=False)

    z_dt = fp8 if QKV_FP8 else bf16
    e_dt = fp8 if PV_FP8 else bf16
    o_dt = fp8 if WO_FP8 else bf16
    f_dt = fp8 if FFN_FP8 else bf16

    xq = nc.dram_tensor("xq", [N_TOK, E], f32, kind="ExternalInput").ap()
    xk = nc.dram_tensor("xk", [M_TOK, E], f32, kind="ExternalInput").ap()
    xv = nc.dram_tensor("xv", [M_TOK, E], f32, kind="ExternalInput").ap()
    # pretiled weights; DoubleRow layout [P, kpair, 2, N], plain [P, kc, N]
    wq = nc.dram_tensor("wq", [P, EC * E], z_dt, kind="ExternalInput").ap()
    wk = nc.dram_tensor("wk", [P, EC * E], z_dt, kind="ExternalInput").ap()
    wv = nc.dram_tensor("wv", [P, EC * E], z_dt, kind="ExternalInput").ap()
    wo = nc.dram_tensor("wo", [P, EC * E], o_dt, kind="ExternalInput").ap()
    w1 = nc.dram_tensor("w1", [P, EC * F], f_dt, kind="ExternalInput").ap()
    w2 = nc.dram_tensor("w2", [P, FC * E], f_dt, kind="ExternalInput").ap()
    outT = nc.dram_tensor("outT", [E, N_TOK], f32, kind="ExternalOutput").ap()

    w1r = w1.rearrange("p (kc f) -> p kc f", kc=EC)   # [128, 4, 2048]
    w2r = w2.rearrange("p (fc e) -> p fc e", fc=FC)   # [128, 16, 512]

    with tile.TileContext(nc) as tc, ExitStack() as ctx:
        consts = ctx.enter_context(tc.tile_pool(name="consts", bufs=1))
        stage = ctx.enter_context(tc.tile_pool(name="stage", bufs=5))
        zstage = ctx.enter_context(tc.tile_pool(name="zstage", bufs=5))
        small = ctx.enter_context(tc.tile_pool(name="small", bufs=4))
        lnst = ctx.enter_context(tc.tile_pool(name="lnst", bufs=6))
        recp = ctx.enter_context(tc.tile_pool(name="recp", bufs=4))
        epool = ctx.enter_context(tc.tile_pool(name="epool", bufs=2))
        opool = ctx.enter_context(tc.tile_pool(name="opool", bufs=2))
        xslot = ctx.enter_context(tc.tile_pool(name="xslot", bufs=3))
        wstr = ctx.enter_context(tc.tile_pool(name="wstr", bufs=2))
        hold = ctx.enter_context(tc.tile_pool(name="hold", bufs=1))
        ps_s = ctx.enter_context(tc.tile_pool(name="ps_s", bufs=3, space="PSUM"))
        ps_acc = ctx.enter_context(tc.tile_pool(name="ps_acc", bufs=2, space="PSUM"))

        # ---------------- constants ----------------
        ident_f = consts.tile([P, P], f32)
        make_identity(nc, ident_f[:])
        ident_z = consts.tile([P, P], z_dt)
        make_identity(nc, ident_z[:])
        ident_o = consts.tile([P, P], o_dt)
        make_identity(nc, ident_o[:])
        ones_b = consts.tile([P, 1], bf16)
        nc.any.memset(ones_b[:], 1.0)
        ones_row = consts.tile([1, P], bf16)
        nc.any.memset(ones_row[:], 1.0)
        eps_b = consts.tile([P, 1], f32)
        nc.any.memset(eps_b[:], EPS)
        shift_b = consts.tile([P, 1], f32)
        nc.any.memset(shift_b[:], EXP_SHIFT)

        # resident weights
        wq_t = consts.tile([P, EC, E], z_dt)
        wk_t = consts.tile([P, EC, E], z_dt)
        wv_t = consts.tile([P, EC, E], z_dt)
        wo_t = consts.tile([P, EC, E], o_dt)
        for dram, sb in ((wq, wq_t), (wk, wk_t), (wv, wv_t)):
            nc.sync.dma_start(sb[:].rearrange("p a b -> p (a b)"), dram[:])

        def dr_w(w_t):   # [P, kc, N] viewed as [P, kpair, 2, N]
            return w_t[:].rearrange("p (j i) n -> p j i n", i=2)

        def dr_z(zT):    # [P, kc, T] viewed as [P, kpair, 2, T]
            return zT[:].rearrange("p (j i) t -> p j i t", i=2)

        for _rep in range(nrep):
            # persistent activations
            zqT = xslot.tile([P, EC, N_TOK], z_dt, tag="x")
            zkT = xslot.tile([P, EC, M_TOK], z_dt, tag="x")
            zvT = xslot.tile([P, EC, M_TOK], z_dt, tag="x")
            xqT = hold.tile([P, EC, N_TOK], f32, tag="xqT")
            qT = hold.tile([P, EC, N_TOK], bf16, tag="qz")
            kT = hold.tile([P, EC, M_TOK], bf16, tag="ko")

            # ---------------- P1: LN1 + transpose to feature-major ----------------
            def ln_transpose(x_dram, tgroups, zT, also_raw_to=None):
                for g in tgroups:
                    zts = []
                    xts = []
                    for t in range(4):
                        row0 = (g * 4 + t) * P
                        xt = stage.tile([P, E], f32, tag="xin")
                        nc.sync.dma_start(xt[:], x_dram[row0:row0 + P, :])
                        stats = small.tile([P, 6], f32, tag="stats")
                        aggr = small.tile([P, 2], f32, tag="aggr")
                        nc.vector.bn_stats(stats[:], xt[:])
                        nc.vector.bn_aggr(aggr[:], stats[:])
                        stdev = small.tile([P, 1], f32, tag="stdev")
                        nc.scalar.activation(stdev[:], aggr[:, 1:2], AF.Sqrt,
                                             bias=eps_b[:], scale=1.0)
                        rstd = small.tile([P, 1], f32, tag="rstd")
                        nc.vector.reciprocal(rstd[:], stdev[:])
                        zt = zstage.tile([P, E], z_dt, tag="zt")
                        nc.vector.tensor_scalar(zt[:], xt[:], aggr[:, 0:1],
                                                rstd[:],
                                                ALU.subtract, ALU.mult)
                        zts.append(zt)
                        xts.append(xt)
                    for c in range(EC):
                        # fp8 transpose requires output element step of 2
                        ptr = ps_s.tile([P, 512, 2], z_dt, tag="s")
                        for t in range(4):
                            nc.tensor.transpose(ptr[:, t * P:(t + 1) * P, 0],
                                                zts[t][:, c * P:(c + 1) * P],
                                                ident_z[:])
                        nc.scalar.copy(zT[:, c, g * 512:(g + 1) * 512],
                                       ptr[:, :, 0])
                    if also_raw_to is not None:
                        for c in range(EC):
                            ptf = ps_s.tile([P, 512], f32, tag="s")
                            for t in range(4):
                                nc.tensor.transpose(ptf[:, t * P:(t + 1) * P],
                                                    xts[t][:, c * P:(c + 1) * P],
                                                    ident_f[:])
                            nc.vector.tensor_copy(
                                also_raw_to[:, c, g * 512:(g + 1) * 512], ptf[:])

            ln_transpose(xq, range(NG), zqT, also_raw_to=xqT)
            ln_transpose(xk, range(MG), zkT)

            # ---------------- P2: QKV projections ----------------
            for zT, w_t, dstT, ngroups in ((zqT, wq_t, qT, NG), (zkT, wk_t, kT, MG)):
                for g in range(ngroups):
                    ts_ = slice(g * 512, (g + 1) * 512)
                    for n in range(EC):
                        pp = ps_acc.tile([P, 512], f32, tag="acc")
                        if QKV_FP8:
                            wdr, zdr = dr_w(w_t), dr_z(zT)
                            for j in range(2):
                                nc.tensor.matmul(pp[:],
                                                 wdr[:, j, :, n * P:(n + 1) * P],
                                                 zdr[:, j, :, ts_],
                                                 start=(j == 0), stop=(j == 1),
                                                 perf_mode=DR)
                        else:
                            for kc in range(EC):
                                nc.tensor.matmul(pp[:],
                                                 w_t[:, kc, n * P:(n + 1) * P],
                                                 zT[:, kc, ts_],
                                                 start=(kc == 0),
                                                 stop=(kc == EC - 1))
                        nc.vector.tensor_copy(dstT[:, n, ts_], pp[:])

            # V -> token-major, augmented with a ones column per head
            # (emitted inside the P3 unit loop to overlap the exp-paced phase)
            def emit_v(v_aug, ms):
                for m in ms:
                    pp = ps_acc.tile([P, 512], f32, tag="acc")
                    if QKV_FP8:
                        zdr = dr_z(zvT)
                        for j in range(2):
                            nc.tensor.matmul(pp[:],
                                             zdr[:, j, :, m * P:(m + 1) * P],
                                             dr_w(wv_t)[:, j, :, :],
                                             start=(j == 0), stop=(j == 1),
                                             perf_mode=DR)
                    else:
                        for kc in range(EC):
                            nc.tensor.matmul(pp[:], zvT[:, kc, m * P:(m + 1) * P],
                                             wv_t[:, kc, :],
                                             start=(kc == 0), stop=(kc == EC - 1))
                    nc.vector.tensor_copy(
                        v_aug[:, m, :, 0:D],
                        pp[:].rearrange("p (h d) -> p h d", h=H))

            # ---------------- P3: attention ----------------
            # per (head-pair, half) unit: scores+exp into e_all, then PV for
            # the previous unit overlaps the next unit's scores on PE.
            O_tok = hold.tile([P, NC, E], o_dt, tag="otok")  # [q, qc, e]
            units = []
            for hp in range(EC):
                units.append((hp, slice(0, 64), 2 * hp))
                units.append((hp, slice(64, 128), 2 * hp + 1))

            i32 = mybir.dt.int32

            def emit_scores_exp(u):
                hp, pr, hidx = u
                e_all = epool.tile([P, MC, N_TOK], e_dt, tag="e",
                                   name=f"e_{hidx}")
                for m in range(MC):
                    ps = ps_s.tile([P, N_TOK], f32, tag="s")
                    for g in range(NG):
                        ts_ = slice(g * 512, (g + 1) * 512)
                        nc.tensor.matmul(ps[:, ts_], kT[pr, hp, m * P:(m + 1) * P],
                                         qT[pr, hp, ts_], start=True, stop=True)
                    if m in FEXP_SLABS:
                        # DVE fast-exp: engine-balance offload from Act;
                        # the fp8 convert-copy runs on Pool
                        fx = hold.tile([P, N_TOK], f32, tag="aT")
                        nc.vector.tensor_scalar(fx[:].bitcast(i32), ps[:],
                                                FEXP_MUL, FEXP_ADD,
                                                ALU.mult, ALU.add)
                        nc.gpsimd.tensor_copy(e_all[:, m, :], fx[:])
                    else:
                        nc.scalar.activation(e_all[:, m, :], ps[:], AF.Exp,
                                             bias=shift_b[:], scale=EXP_SCALE)
                return e_all

            def emit_pv(u, e_all):
                hp, pr, hidx = u
                for qc in range(NC):
                    acc = ps_acc.tile([P, D + 1], f32, tag="acc")
                    qs = slice(qc * P, (qc + 1) * P)
                    if PV_FP8:
                        edr = e_all[:].rearrange("p (j i) t -> p j i t", i=2)
                        vdr = v_aug[:].rearrange("p (j i) h d -> p j i h d", i=2)
                        for j in range(MC // 2):
                            nc.tensor.matmul(acc[:], edr[:, j, :, qs],
                                             vdr[:, j, :, hidx, :],
                                             start=(j == 0),
                                             stop=(j == MC // 2 - 1),
                                             perf_mode=DR)
                    else:
                        for m in range(MC):
                            nc.tensor.matmul(acc[:], e_all[:, m, qs],
                                             v_aug[:, m, hidx, :],
                                             start=(m == 0), stop=(m == MC - 1))
                    rec = recp.tile([P, 1], f32, tag="rec")
                    nc.vector.reciprocal(rec[:], acc[:, D:D + 1])
                    nc.vector.tensor_scalar(
                        O_tok[:, qc, hidx * D:(hidx + 1) * D],
                        acc[:, 0:D], rec[:], None, ALU.mult)

            ln_transpose(xv, range(MG), zvT)
            nc.sync.dma_start(wo_t[:].rearrange("p a b -> p (a b)"), wo[:])
            v_aug = xslot.tile([P, MC, H, D + 1], e_dt, tag="x")
            nc.any.memset(v_aug[:, :, :, D:D + 1], 1.0)
            emit_v(v_aug, range(MC))
            prev = None
            for ui, u in enumerate(units):
                e_all = emit_scores_exp(u)
                if prev is not None:
                    emit_pv(*prev)
                prev = (u, e_all)
            emit_pv(*prev)

            # transpose O to feature-major
            oT = hold.tile([P, EC, N_TOK], o_dt, tag="oT")
            for ec in range(EC):
                pt = ps_acc.tile([P, N_TOK, 2], o_dt, tag="acc")
                for qc in range(NC):
                    nc.tensor.transpose(pt[:, qc * P:(qc + 1) * P, 0],
                                        O_tok[:, qc, ec * P:(ec + 1) * P],
                                        ident_o[:])
                nc.vector.tensor_copy(oT[:, ec, :], pt[:, :, 0])

            # ---------------- P4: Wo projection + residual ----------------
            aT = hold.tile([P, EC, N_TOK], f32, tag="aT")
            for g in range(NG):
                ts_ = slice(g * 512, (g + 1) * 512)
                for e in range(EC):
                    pp = ps_acc.tile([P, 512], f32, tag="acc")
                    if WO_FP8:
                        for j in range(2):
                            nc.tensor.matmul(pp[:],
                                             dr_w(wo_t)[:, j, :, e * P:(e + 1) * P],
                                             dr_z(oT)[:, j, :, ts_],
                                             start=(j == 0), stop=(j == 1),
                                             perf_mode=DR)
                    else:
                        for kc in range(EC):
                            nc.tensor.matmul(pp[:], wo_t[:, kc, e * P:(e + 1) * P],
                                             oT[:, kc, ts_],
                                             start=(kc == 0), stop=(kc == EC - 1))
                    nc.vector.tensor_add(aT[:, e, ts_], pp[:], xqT[:, e, ts_])

            # ---------------- P5: LN2 stats ----------------
            a2pool = ctx.enter_context(tc.tile_pool(name="a2pool", bufs=1))
            z2T = hold.tile([P, EC, N_TOK], f_dt, tag="ko")  # reuses kT slot
            for g in range(NG):
                ts_ = slice(g * 512, (g + 1) * 512)
                abf_g = a2pool.tile([P, EC, 512], bf16, tag="abf")
                a2_g = a2pool.tile([P, EC, 512], bf16, tag="a2")
                for e in range(EC):
                    nc.gpsimd.tensor_copy(abf_g[:, e, :], aT[:, e, ts_])
                    nc.vector.tensor_mul(a2_g[:, e, :], aT[:, e, ts_],
                                         aT[:, e, ts_])
                st = ps_acc.tile([P, 512], f32, tag="acc")
                for kc in range(EC):
                    nc.tensor.matmul(st[0:1, :], ones_b[:], abf_g[:, kc, :],
                                     start=(kc == 0), stop=(kc == EC - 1))
                for kc in range(EC):
                    nc.tensor.matmul(st[32:33, :], ones_b[:], a2_g[:, kc, :],
                                     start=(kc == 0), stop=(kc == EC - 1),
                                     tile_position=(0, 32),
                                     skip_group_check=True)
                mu = lnst.tile([1, 512], f32, tag="ln2")
                msq = lnst.tile([1, 512], f32, tag="ln2")
                nc.scalar.mul(mu[:], st[0:1, :], 1.0 / E)
                nc.scalar.mul(msq[:], st[32:33, :], 1.0 / E)
                var = lnst.tile([1, 512], f32, tag="ln2")
                nc.vector.tensor_mul(var[:], mu[:], mu[:])
                nc.vector.tensor_tensor(var[:], msq[:], var[:], ALU.subtract)
                stdev = lnst.tile([1, 512], f32, tag="ln2")
                nc.scalar.activation(stdev[:], var[:], AF.Sqrt, bias=eps_b[0:1, :],
                                     scale=1.0)
                rstd = lnst.tile([1, 512], f32, tag="ln2")
                nc.vector.reciprocal(rstd[:], stdev[:])
                mr = lnst.tile([1, 512], f32, tag="ln2")
                nc.vector.tensor_mul(mr[:], mu[:], rstd[:])
                rstd16 = lnst.tile([1, 512], bf16, tag="ln2b")
                mr16 = lnst.tile([1, 512], bf16, tag="ln2b")
                nc.vector.tensor_copy(rstd16[:], rstd[:])
                nc.vector.tensor_copy(mr16[:], mr[:])
                bc_r = ps_acc.tile([P, 512], f32, tag="acc")
                bc_m = ps_acc.tile([P, 512], f32, tag="acc")
                nc.tensor.matmul(bc_r[:], ones_row[:], rstd16[:],
                                 start=True, stop=True)
                nc.tensor.matmul(bc_m[:], ones_row[:], mr16[:],
                                 start=True, stop=True)
                for e in range(EC):
                    tmp = opool.tile([P, 512], f32, tag="ot")
                    nc.vector.tensor_mul(tmp[:], aT[:, e, ts_], bc_r[:])
                    nc.vector.tensor_tensor(z2T[:, e, ts_], tmp[:],
                                            bc_m[:], ALU.subtract)

            # ---------------- P6: FFN1 + gelu ----------------
            h1 = xslot.tile([P, FC // 2, N_TOK], f_dt, tag="x")
            h2 = xslot.tile([P, FC // 2, N_TOK], f_dt, tag="x")
            hs = (h1, h2)
            for f in range(FC):
                w1f = wstr.tile([P, EC, P], f_dt, tag="w1f")
                nc.sync.dma_start(w1f[:], w1r[:, :, f * P:(f + 1) * P])
                pp = ps_s.tile([P, N_TOK], f32, tag="s")
                for g in range(NG):
                    ts_ = slice(g * 512, (g + 1) * 512)
                    for kc in range(EC):
                        nc.tensor.matmul(pp[:, ts_],
                                         w1f[:, kc, :],
                                         z2T[:, kc, ts_],
                                         start=(kc == 0), stop=(kc == EC - 1))
                nc.scalar.activation(hs[f // 8][:, f % 8, :], pp[:], AF.Gelu,
                                     bias=0.0, scale=1.0)

            # ---------------- P7: FFN2 + residual -> outT ----------------
            for e in range(EC):
                w2e = wstr.tile([P, FC, P], f_dt, tag="w2e")
                nc.sync.dma_start(w2e[:], w2r[:, :, e * P:(e + 1) * P])
                for g in range(NG):
                    ts_ = slice(g * 512, (g + 1) * 512)
                    pp = ps_acc.tile([P, 512], f32, tag="acc")
                    for f in range(FC):
                        nc.tensor.matmul(pp[:],
                                         w2e[:, f, :],
                                         hs[f // 8][:, f % 8, ts_],
                                         start=(f == 0), stop=(f == FC - 1))
                    ot = opool.tile([P, 512], f32, tag="ot")
                    nc.vector.tensor_add(ot[:], pp[:], aT[:, e, ts_])
                    nc.sync.dma_start(outT[e * P:(e + 1) * P, ts_], ot[:])

    nc.compile()
    return nc


def _np_dt(dt_):
    return {fp8: ml_dtypes.float8_e4m3, bf16: ml_dtypes.bfloat16}[dt_]


def _tile_w(w, kc, dt_):
    """[K, N] -> [128, kc*N]: w[p, c*N+n] = W[c*128+p, n]."""
    K, N = w.shape
    assert K == kc * P
    return np.ascontiguousarray(
        w.reshape(kc, P, N).transpose(1, 0, 2).reshape(P, kc * N)
    ).astype(_np_dt(dt_))


def make_in_maps(inputs):
    xq = np.asarray(inputs["xq"], np.float32)
    xk = np.asarray(inputs["xk"], np.float32)
    xv = np.asarray(inputs["xv"], np.float32)
    Wq, bq = np.asarray(inputs["Wq"], np.float32), np.asarray(inputs["bq"], np.float32)
    Wk, bk = np.asarray(inputs["Wk"], np.float32), np.asarray(inputs["bk"], np.float32)
    Wv, bv = np.asarray(inputs["Wv"], np.float32), np.asarray(inputs["bv"], np.float32)
    Wo = np.asarray(inputs["Wo"], np.float32)
    g1, b1 = np.asarray(inputs["g1"], np.float32), np.asarray(inputs["b1"], np.float32)
    g2, b2 = np.asarray(inputs["g2"], np.float32), np.asarray(inputs["b2"], np.float32)
    W_ff1 = np.asarray(inputs["W_ff1"], np.float32)
    b_ff1 = np.asarray(inputs["b_ff1"], np.float32)
    W_ff2 = np.asarray(inputs["W_ff2"], np.float32)
    b_ff2 = np.asarray(inputs["b_ff2"], np.float32)

    Wq_eff = g1[:, None] * Wq  # 1/sqrt(D) applied via exp scale on-device
    Wk_eff = g1[:, None] * Wk
    Wv_eff = g1[:, None] * Wv
    W1_eff = g2[:, None] * W_ff1
    for name, vec in (("cbq", b1 @ Wq + bq), ("cbk", b1 @ Wk + bk),
                      ("cbv", b1 @ Wv + bv), ("cb1", b2 @ W_ff1 + b_ff1),
                      ("cb2", b_ff2)):
        if not np.allclose(vec, 0.0, atol=1e-6):
            raise NotImplementedError(f"nonzero folded bias {name}")

    z_dt = fp8 if QKV_FP8 else bf16
    o_dt = fp8 if WO_FP8 else bf16
    f_dt = fp8 if FFN_FP8 else bf16
    w_maps = {
        "wq": _tile_w(Wq_eff, EC, z_dt), "wk": _tile_w(Wk_eff, EC, z_dt),
        "wv": _tile_w(Wv_eff, EC, z_dt), "wo": _tile_w(Wo, EC, o_dt),
        "w1": _tile_w(W1_eff, EC, f_dt), "w2": _tile_w(W_ff2, FC, f_dt),
    }
    return [
        {"xq": np.ascontiguousarray(xq[b]), "xk": np.ascontiguousarray(xk[b]),
         "xv": np.ascontiguousarray(xv[b]), **w_maps}
        for b in range(N_CORES)
    ]


def run(inputs, trace=False):
    global _PROGRAM
    in_maps = make_in_maps(inputs)
    if _PROGRAM is None:
        _PROGRAM = _build_program()
    res = run_bass_kernel_spmd(_PROGRAM, in_maps, list(range(N_CORES)),
                               trace=trace)
    out = np.stack([np.ascontiguousarray(r["outT"].T.astype(np.float32))
                    for r in res.results])
    return out, res.exec_time_ns


def kernel(**inputs):
    out, _ = run(inputs, trace=False)
    return out
